# revision 5
# baseline (speedup 1.0000x reference)
"""Pure-fp16 variant: 2 bytes/element (vs 3 for the fp16+fp8-residual
baseline) -> ~33% less HBM traffic, which is the binding constraint
(per-core DMA roofline ~358 GB/s; 25.2 MB/core fp16 -> ~70 us floor).

Accuracy: X~N(0,1), W~N(0,0.01^2); fp16 quantization of both inputs with
exact products accumulated in fp32 PSUM gives max rel err ~3.3e-4
(validated in numpy) vs the 2e-2 gate -- no residual correction needed.
This also drops the 2 extra residual matmuls per k-block (3x less PE
work) and all DVE upconverts.

Layout: A and W are packed per (patch, k-block) into one DRAM tensor
G[KP=128, PPC, KC, 64+128] fp16 so each patch-group loads with a single
large contiguous DMA (gp=2 -> 1.57 MB, 12 KB contiguous per partition).
Groups alternate between the two HWDGE rings (sync/scalar); outputs
stream back in fp16 over SWDGE (gpsimd) to stay off the input rings.
"""

from contextlib import ExitStack

import numpy as np

N_CORES = 8
N, H, W_IMG, FIN = 64, 128, 128, 32
FH = FW = 8
FOUT = 128
NR, NCOL = H // FH, W_IMG // FW
P = NR * NCOL
PPC = P // N_CORES
K = FH * FW * FIN
KP = 128
KC = K // KP
GW = N + FOUT  # packed inner width: [0:N]=A block, [N:]=W block

_PROGRAM_CACHE = {}


def build_program(bufs=6):
    import concourse.mybir as mybir
    import concourse.tile as tile
    from concourse import bacc

    nc = bacc.Bacc()
    f16 = mybir.dt.float16
    f32 = mybir.dt.float32
    g_d = nc.dram_tensor("G", [KP, PPC, KC, GW], f16, kind="ExternalInput")
    b_d = nc.dram_tensor("bias", [FOUT], f32, kind="ExternalInput")
    z_d = nc.dram_tensor("Z", [FOUT, PPC, N], f16, kind="ExternalOutput")

    with tile.TileContext(nc) as tc, ExitStack() as ctx:
        gpool = ctx.enter_context(tc.tile_pool(name="g", bufs=bufs))
        opool = ctx.enter_context(tc.tile_pool(name="o", bufs=4))
        pspool = ctx.enter_context(tc.tile_pool(name="ps", bufs=4, space="PSUM"))
        singles = ctx.enter_context(tc.tile_pool(name="singles", bufs=1))

        bias_sb = singles.tile([FOUT, 1], f32)
        nc.gpsimd.dma_start(out=bias_sb, in_=b_d[:, None])

        # Input: [2]*14 + [1]*4 alternating sync/scalar (1.57 MB DMAs kept
        # every ring busy at ~343 GB/s in profiling; 3.1 MB groups measured
        # slower). The final patch loads as two half-K DMAs, one per ring,
        # so both rings finish together and its matmuls start sooner.
        group_sizes = [2] * 14 + [1] * 4
        p0 = 0
        for gi, gp in enumerate(group_sizes):
            g = gpool.tile([KP, gp, KC, GW], f16, tag="g")
            eng = nc.sync if gi % 2 == 0 else nc.scalar
            last = gi == len(group_sizes) - 1
            if last:
                nc.scalar.dma_start(
                    out=g[:, :, 0 : KC // 2, :], in_=g_d[:, p0 : p0 + gp, 0 : KC // 2]
                )
                nc.sync.dma_start(
                    out=g[:, :, KC // 2 :, :], in_=g_d[:, p0 : p0 + gp, KC // 2 :]
                )
            else:
                eng.dma_start(out=g, in_=g_d[:, p0 : p0 + gp])

            # One PSUM region + one fused DVE relu(x+bias) + one output DMA
            # per group. Epilogue on DVE so Sync/Scalar stay pure DMA
            # issuers -- an ACT epilogue would stall the scalar ring's next
            # input-DMA issue behind its PSUM wait.
            psum = pspool.tile([FOUT, gp, N], f32, tag="ps")
            for j in range(gp):
                for kc in range(KC):
                    nc.tensor.matmul(
                        psum[:, j, :],
                        g[:, j, kc, N:GW],
                        g[:, j, kc, 0:N],
                        start=(kc == 0),
                        stop=(kc == KC - 1),
                    )
            ot = opool.tile([FOUT, gp, N], f16, tag="ot")
            nc.vector.tensor_scalar(
                ot,
                psum,
                bias_sb,
                0.0,
                op0=mybir.AluOpType.add,
                op1=mybir.AluOpType.max,
            )
            # tail outputs ride the HWDGE rings (idle by then; avoids
            # queueing behind earlier outs on the SWDGE Q7).
            if gi >= len(group_sizes) - 2:
                out_eng = nc.scalar if last else nc.sync
            else:
                out_eng = nc.gpsimd
            out_eng.dma_start(out=z_d[:, p0 : p0 + gp, :], in_=ot)
            p0 += gp
    nc.finalize()
    return nc


def shard_inputs(X, filters, bias):
    X = np.asarray(X, dtype=np.float32)
    filters = np.asarray(filters, dtype=np.float32)
    bias = np.ascontiguousarray(np.asarray(bias, dtype=np.float32))

    xr = X.reshape(N, NR, FH, NCOL, FW, FIN).astype(np.float16)
    xp = xr.transpose(1, 3, 2, 4, 5, 0).reshape(P, K, N)
    a_all = xp.reshape(N_CORES, PPC, KC, KP, N).transpose(0, 3, 1, 2, 4)

    wp = filters.astype(np.float16).reshape(P, K, FOUT)
    w_all = wp.reshape(N_CORES, PPC, KC, KP, FOUT).transpose(0, 3, 1, 2, 4)

    g_all = np.concatenate([a_all, w_all], axis=-1)  # [cores, KP, PPC, KC, GW]
    return [
        {"G": np.ascontiguousarray(g_all[c]), "bias": bias} for c in range(N_CORES)
    ]


def gather_output(per_core_z):
    z = np.stack([np.asarray(zc, dtype=np.float32) for zc in per_core_z], axis=0)
    z = z.transpose(3, 0, 2, 1).reshape(N, P, FOUT)
    return np.ascontiguousarray(z.reshape(N, NR, NCOL, FOUT))


def kernel(X, filters, bias):
    from concourse.bass_utils import run_bass_kernel_spmd

    if "nc" not in _PROGRAM_CACHE:
        _PROGRAM_CACHE["nc"] = build_program()
    nc = _PROGRAM_CACHE["nc"]

    in_maps = shard_inputs(X, filters, bias)
    res = run_bass_kernel_spmd(nc, in_maps, core_ids=list(range(N_CORES)))
    return gather_output([res.results[c]["Z"] for c in range(N_CORES)])


# revision 7
# speedup vs baseline: 1.0061x; 1.0061x over previous
"""Pure-fp16 variant: 2 bytes/element (vs 3 for the fp16+fp8-residual
baseline) -> ~33% less HBM traffic, which is the binding constraint
(per-core DMA roofline ~358 GB/s; 25.2 MB/core fp16 -> ~70 us floor).

Accuracy: X~N(0,1), W~N(0,0.01^2); fp16 quantization of both inputs with
exact products accumulated in fp32 PSUM gives max rel err ~3.3e-4
(validated in numpy) vs the 2e-2 gate -- no residual correction needed.
This also drops the 2 extra residual matmuls per k-block (3x less PE
work) and all DVE upconverts.

Layout: A and W are packed per (patch, k-block) into one DRAM tensor
G[KP=128, PPC, KC, 64+128] fp16 so each patch-group loads with a single
large contiguous DMA (gp=2 -> 1.57 MB, 12 KB contiguous per partition).
Groups alternate between the two HWDGE rings (sync/scalar); outputs
stream back in fp16 over SWDGE (gpsimd) to stay off the input rings.
"""

from contextlib import ExitStack

import numpy as np

N_CORES = 8
N, H, W_IMG, FIN = 64, 128, 128, 32
FH = FW = 8
FOUT = 128
NR, NCOL = H // FH, W_IMG // FW
P = NR * NCOL
PPC = P // N_CORES
K = FH * FW * FIN
KP = 128
KC = K // KP
GW = N + FOUT  # packed inner width: [0:N]=A block, [N:]=W block

_PROGRAM_CACHE = {}


def build_program(bufs=6):
    import concourse.mybir as mybir
    import concourse.tile as tile
    from concourse import bacc

    nc = bacc.Bacc()
    f16 = mybir.dt.float16
    f32 = mybir.dt.float32
    g_d = nc.dram_tensor("G", [KP, PPC, KC, GW], f16, kind="ExternalInput")
    b_d = nc.dram_tensor("bias", [FOUT], f32, kind="ExternalInput")
    z_d = nc.dram_tensor("Z", [FOUT, PPC, N], f16, kind="ExternalOutput")

    with tile.TileContext(nc) as tc, ExitStack() as ctx:
        gpool = ctx.enter_context(tc.tile_pool(name="g", bufs=bufs))
        opool = ctx.enter_context(tc.tile_pool(name="o", bufs=4))
        pspool = ctx.enter_context(tc.tile_pool(name="ps", bufs=4, space="PSUM"))
        singles = ctx.enter_context(tc.tile_pool(name="singles", bufs=1))

        bias_sb = singles.tile([FOUT, 1], f32)
        nc.gpsimd.dma_start(out=bias_sb, in_=b_d[:, None])

        # Input: [2]*14 + [1]*4 alternating sync/scalar (1.57 MB DMAs kept
        # every ring busy at ~343 GB/s in profiling; 3.1 MB groups measured
        # slower). The final patch loads as two half-K DMAs, one per ring,
        # so both rings finish together and its matmuls start sooner.
        group_sizes = [2] * 14 + [1] * 4
        p0 = 0
        for gi, gp in enumerate(group_sizes):
            g = gpool.tile([KP, gp, KC, GW], f16, tag="g")
            last = gi == len(group_sizes) - 1
            # third input queue: two mid-stream groups ride SWDGE (gpsimd)
            # to probe whether the 2 HWDGE rings or the HBM port binds.
            if gi in (4, 9):
                eng = nc.gpsimd
            else:
                eng = nc.sync if gi % 2 == 0 else nc.scalar
            eng.dma_start(out=g, in_=g_d[:, p0 : p0 + gp])

            # One PSUM region + one fused DVE relu(x+bias) + one output DMA
            # per group. Epilogue on DVE so Sync/Scalar stay pure DMA
            # issuers -- an ACT epilogue would stall the scalar ring's next
            # input-DMA issue behind its PSUM wait.
            psum = pspool.tile([FOUT, gp, N], f32, tag="ps")
            for j in range(gp):
                for kc in range(KC):
                    nc.tensor.matmul(
                        psum[:, j, :],
                        g[:, j, kc, N:GW],
                        g[:, j, kc, 0:N],
                        start=(kc == 0),
                        stop=(kc == KC - 1),
                    )
            ot = opool.tile([FOUT, gp, N], f16, tag="ot")
            nc.vector.tensor_scalar(
                ot,
                psum,
                bias_sb,
                0.0,
                op0=mybir.AluOpType.add,
                op1=mybir.AluOpType.max,
            )
            # tail outputs ride the HWDGE rings (idle by then; avoids
            # queueing behind earlier outs on the SWDGE Q7).
            if gi >= len(group_sizes) - 2:
                out_eng = nc.scalar if last else nc.sync
            else:
                out_eng = nc.gpsimd
            out_eng.dma_start(out=z_d[:, p0 : p0 + gp, :], in_=ot)
            p0 += gp
    nc.finalize()
    return nc


def shard_inputs(X, filters, bias):
    X = np.asarray(X, dtype=np.float32)
    filters = np.asarray(filters, dtype=np.float32)
    bias = np.ascontiguousarray(np.asarray(bias, dtype=np.float32))

    xr = X.reshape(N, NR, FH, NCOL, FW, FIN).astype(np.float16)
    xp = xr.transpose(1, 3, 2, 4, 5, 0).reshape(P, K, N)
    a_all = xp.reshape(N_CORES, PPC, KC, KP, N).transpose(0, 3, 1, 2, 4)

    wp = filters.astype(np.float16).reshape(P, K, FOUT)
    w_all = wp.reshape(N_CORES, PPC, KC, KP, FOUT).transpose(0, 3, 1, 2, 4)

    g_all = np.concatenate([a_all, w_all], axis=-1)  # [cores, KP, PPC, KC, GW]
    return [
        {"G": np.ascontiguousarray(g_all[c]), "bias": bias} for c in range(N_CORES)
    ]


def gather_output(per_core_z):
    z = np.stack([np.asarray(zc, dtype=np.float32) for zc in per_core_z], axis=0)
    z = z.transpose(3, 0, 2, 1).reshape(N, P, FOUT)
    return np.ascontiguousarray(z.reshape(N, NR, NCOL, FOUT))


def kernel(X, filters, bias):
    from concourse.bass_utils import run_bass_kernel_spmd

    if "nc" not in _PROGRAM_CACHE:
        _PROGRAM_CACHE["nc"] = build_program()
    nc = _PROGRAM_CACHE["nc"]

    in_maps = shard_inputs(X, filters, bias)
    res = run_bass_kernel_spmd(nc, in_maps, core_ids=list(range(N_CORES)))
    return gather_output([res.results[c]["Z"] for c in range(N_CORES)])
